# revision 9
# baseline (speedup 1.0000x reference)
"""GATv2 GNN classifier (nn_AttGNNClassifier) as an 8-core Trainium2 Bass kernel.

Strategy (graph-parallel):
  - Nodes are partitioned contiguously across 8 cores (NPC nodes/core, padded).
  - Edges are assigned to the core owning their dst node, grouped into tiles of
    128 dst nodes, and padded to a uniform per-tile edge count (ET = EC*128).
  - Per layer: each core projects its own nodes (fs = h @ Ws, fd = h @ Wd),
    all-gathers fs into a full table (rows padded to 256 fp16 = 512B for the
    gather engine), then processes its edge tiles:
      * dma_gather fs[src] rows (edge-major, int16 indices, split into two
        sections so indices stay < 32768)
      * fd[dst] broadcast to edges via a node-major one-hot matmul, fs added
        into the same PSUM via an identity matmul -> z = fs+fd in PSUM
      * leaky_relu, attention logits via a-mult + head-reduce, exp on ACT,
        and a one-hot (dst == node) matmul aggregation computing unnormalized
        sums and softmax denominators in one PSUM pass.
      * finalize: h = u/denom + bias (head-mean on the last layer).
  - Graph mean-pool via a one-hot (graph_id == g) matmul accumulated across
    tiles, an all-reduce of [G, 65] partials, then the tiny pattern/classifier
    MLP on every core; core 0's output is returned.

Everything data-dependent (edge sorting, padding, index layouts) is prepared on
the host inside kernel() before compiling; the device program is static.
"""

import math

import numpy as np

import concourse.bass as bass
import concourse.bacc as bacc
import concourse.mybir as mybir
import concourse.tile as tile
from concourse import library_config
from concourse.bass_utils import run_bass_kernel_spmd

F16 = mybir.dt.float16
F32 = mybir.dt.float32
I16 = mybir.dt.int16

NEG_GAT = 0.2
NEG = 0.01


def _default_cfg():
    return dict(
        NC=8, N=50000, E=400000, F_IN=128, H=3, D=64, G=64, P=64, SPLIT=32768,
    )


def _derive(cfg):
    c = dict(cfg)
    c["HD"] = c["H"] * c["D"]
    c["TE"] = int(math.ceil(c["HD"] / 128)) * 128  # table row elems (512B rows)
    c["NPC"] = int(math.ceil(c["N"] / c["NC"] / 128)) * 128
    c["NPAD"] = c["NC"] * c["NPC"]
    c["NT"] = c["NPC"] // 128
    fc = []
    off = 0
    while off < c["HD"]:
        sz = min(128, c["HD"] - off)
        fc.append((off, sz))
        off += sz
    c["FCH"] = fc
    assert c["F_IN"] <= 128
    return c


# ---------------------------------------------------------------- host prep

def _wrap16(vals, F):
    """int16 values -> [128, F] wrapped (k -> [k%16, k//16]) x8 replicated."""
    out = np.zeros((128, F), np.int16)
    k = np.arange(len(vals))
    out[k % 16, k // 16] = vals
    for g in range(1, 8):
        out[16 * g : 16 * g + 16] = out[:16]
    return out


def prep_host(inputs, cfg):
    c = cfg
    NC, N, NPC, NT, G = c["NC"], c["N"], c["NPC"], c["NT"], c["G"]
    H, D, HD, F_IN, TE, SPLIT = c["H"], c["D"], c["HD"], c["F_IN"], c["TE"], c["SPLIT"]

    src = np.asarray(inputs["src"]).astype(np.int64)
    dst = np.asarray(inputs["dst"]).astype(np.int64)
    graph_ids = np.asarray(inputs["graph_ids"]).astype(np.int64)
    x = np.asarray(inputs["inputs"]).astype(np.float32)

    owner = dst // NPC
    dstl = dst - owner * NPC
    tile_id = dstl // 128
    dst_rel = dstl % 128
    sect = (src >= SPLIT).astype(np.int64)  # 0 = A, 1 = B

    key = (owner * NT + tile_id) * 2 + sect
    order = np.argsort(key, kind="stable")
    cnt = np.bincount(key[order], minlength=NC * NT * 2).reshape(NC, NT, 2)
    KA = int(math.ceil(max(1, cnt[:, :, 0].max()) / 128)) * 128
    KBraw = int(cnt[:, :, 1].max())
    KB = int(math.ceil(KBraw / 128)) * 128 if KBraw > 0 else 0
    ET = KA + KB
    EC = ET // 128

    dstr_f = np.full((NC, NT, 128, EC), 200.0, np.float32)
    dstr_row = np.full((NC, NT, ET), 200.0, np.float32)
    idxA = np.zeros((NC, NT, KA), np.int64)
    idxB = np.zeros((NC, NT, KB), np.int64) if KB else None

    starts = np.concatenate([[0], np.cumsum(cnt.reshape(-1))]).astype(np.int64)
    for core in range(NC):
        for t in range(NT):
            for s in range(2):
                k = (core * NT + t) * 2 + s
                lo, hi = starts[k], starts[k + 1]
                e = order[lo:hi]
                n = hi - lo
                if n == 0:
                    continue
                base = 0 if s == 0 else KA
                sl = base + np.arange(n)
                p, j = sl % 128, sl // 128
                dstr_f[core, t, p, j] = dst_rel[e]
                dstr_row[core, t, sl] = dst_rel[e]
                if s == 0:
                    idxA[core, t, :n] = src[e]
                else:
                    idxB[core, t, :n] = src[e] - SPLIT

    FA = KA // 16
    idxA_rep = np.zeros((NC, 128, NT * FA), np.int16)
    for core in range(NC):
        for t in range(NT):
            idxA_rep[core, :, t * FA : (t + 1) * FA] = _wrap16(idxA[core, t], FA)
    idxB_rep = None
    if KB:
        FB = KB // 16
        idxB_rep = np.zeros((NC, 128, NT * FB), np.int16)
        for core in range(NC):
            for t in range(NT):
                idxB_rep[core, :, t * FB : (t + 1) * FB] = _wrap16(idxB[core, t], FB)

    x_fm = np.zeros((NC, F_IN, NPC), np.float16)
    for core in range(NC):
        lo, hi = core * NPC, min(N, (core + 1) * NPC)
        if hi > lo:
            x_fm[core, :, : hi - lo] = x[lo:hi].T.astype(np.float16)

    gid_f = np.full((NC, NT, 128), -1.0, np.float32)
    for core in range(NC):
        lo, hi = core * NPC, min(N, (core + 1) * NPC)
        if hi > lo:
            flat = np.full(NPC, -1.0, np.float32)
            flat[: hi - lo] = graph_ids[lo:hi].astype(np.float32)
            gid_f[core] = flat.reshape(NT, 128)

    rep = lambda v, p=128: np.broadcast_to(
        np.asarray(v, np.float16)[None, :], (p, len(v))
    ).copy()

    def w16(k):
        return np.asarray(inputs[k]).astype(np.float16)

    def ws_pad(k):  # pad output cols HD -> TE
        w = np.asarray(inputs[k]).astype(np.float16)
        out = np.zeros((w.shape[0], TE), np.float16)
        out[:, :HD] = w
        return out

    a_flat = [np.asarray(inputs[f"a{l}"]).astype(np.float32).reshape(-1) for l in (1, 2, 3)]
    b_flat = [np.asarray(inputs[f"b{l}"]).astype(np.float32) for l in (1, 2, 3)]
    b3m = b_flat[2].reshape(H, D).mean(0)

    iota128 = np.broadcast_to(np.arange(128, dtype=np.float16)[None, :], (128, 128)).copy()
    piota = np.arange(128, dtype=np.float32)[:, None].copy()
    giota = np.broadcast_to(np.arange(G, dtype=np.float16)[None, :], (128, G)).copy()
    ident = np.eye(128, dtype=np.float16)

    bex = np.asarray(inputs["bex"]).astype(np.float32)
    bex96 = np.concatenate([bex, bex, bex])

    common = dict(
        W1s=ws_pad("W1s"), W1d=w16("W1d"),
        W2s=ws_pad("W2s"), W2d=w16("W2d"),
        W3s=ws_pad("W3s"), W3d=w16("W3d"),
        a1_rep=rep(a_flat[0]), a2_rep=rep(a_flat[1]), a3_rep=rep(a_flat[2]),
        b1_rep=rep(b_flat[0]), b2_rep=rep(b_flat[1]),
        b3m_rep=rep(b3m),
        iota=iota128, piota=piota, giota=giota, ident=ident,
        p1T=w16("p1").T.copy(), p2T=w16("p2").T.copy(), p3T=w16("p3").T.copy(),
        Wex=w16("Wex"), bex96_rep=rep(bex96, G),
        Wpat=w16("Wpat"), bpat_rep=rep(np.asarray(inputs["bpat"], np.float32), G),
        Wc1=w16("Wc1"), bc1_rep=rep(np.asarray(inputs["bc1"], np.float32), G),
        Wc2=w16("Wc2"), bc2_rep=rep(np.asarray(inputs["bc2"], np.float32), G),
        Wc3=w16("Wc3"), bc3_rep=rep(np.asarray(inputs["bc3"], np.float32), G),
    )

    in_maps = []
    for core in range(NC):
        m = dict(common)
        m["x_fm"] = x_fm[core]
        m["idxA"] = idxA_rep[core]
        if KB:
            m["idxB"] = idxB_rep[core]
        m["dstr_f"] = dstr_f[core]
        m["dstr_row"] = dstr_row[core]
        m["gid_f"] = gid_f[core][:, :, None]
        in_maps.append(m)
    return in_maps, (KA, KB)


# ---------------------------------------------------------------- device build

def build_gat(cfg, KA, KB):
    c = cfg
    NC, NPC, NPAD, NT, G = c["NC"], c["NPC"], c["NPAD"], c["NT"], c["G"]
    H, D, HD, F_IN, TE, SPLIT = c["H"], c["D"], c["HD"], c["F_IN"], c["TE"], c["SPLIT"]
    FCH = c["FCH"]
    ET = KA + KB
    EC, ECA = ET // 128, KA // 128
    FA, FB = KA // 16, KB // 16

    nc = bacc.Bacc("TRN2", target_bir_lowering=False, debug=False, num_devices=NC)

    def din(name, shape, dt=F16):
        return nc.dram_tensor(name, shape, dt, kind="ExternalInput")

    x_fm = din("x_fm", [F_IN, NPC])
    idxA = din("idxA", [128, NT * FA], I16)
    idxB = din("idxB", [128, NT * FB], I16) if KB else None
    dstr_f = din("dstr_f", [NT, 128, EC], F32)
    dstr_row = din("dstr_row", [NT, ET], F32)
    gid_f = din("gid_f", [NT, 128, 1], F32)

    Wmat = {
        1: (din("W1s", [F_IN, TE]), din("W1d", [F_IN, HD])),
        2: (din("W2s", [HD, TE]), din("W2d", [HD, HD])),
        3: (din("W3s", [HD, TE]), din("W3d", [HD, HD])),
    }
    a_rep = {l: din(f"a{l}_rep", [128, HD]) for l in (1, 2, 3)}
    b_rep = {1: din("b1_rep", [128, HD]), 2: din("b2_rep", [128, HD])}
    b3m_rep = din("b3m_rep", [128, D])
    iota = din("iota", [128, 128])
    piota = din("piota", [128, 1], F32)
    giota = din("giota", [128, G])
    ident = din("ident", [128, 128])
    p123T = [din("p1T", [64, G]), din("p2T", [64, G]), din("p3T", [64, G])]
    Wex = din("Wex", [64, 32])
    bex96_rep = din("bex96_rep", [G, 96])
    Wpat = din("Wpat", [96, 64])
    bpat_rep = din("bpat_rep", [G, 64])
    Wc1 = din("Wc1", [128, 64])
    bc1_rep = din("bc1_rep", [G, 64])
    Wc2 = din("Wc2", [64, 32])
    bc2_rep = din("bc2_rep", [G, 32])
    Wc3 = din("Wc3", [32, 2])
    bc3_rep = din("bc3_rep", [G, 2])

    out = nc.dram_tensor("out", [G, 2], F32, kind="ExternalOutput")

    fs_own = {l: nc.dram_tensor(f"fs_own{l}", [NPC, TE], F16) for l in (1, 2, 3)}
    fs_full = {
        l: nc.dram_tensor(f"fs_full{l}", [NPAD, TE], F16, addr_space="Shared")
        for l in (1, 2, 3)
    }
    partials = nc.dram_tensor("partials", [G, 65], F32)
    partials_red = nc.dram_tensor("partials_red", [G, 65], F32, addr_space="Shared")

    groups = [list(range(NC))]

    with tile.TileContext(nc) as tc:
        with (
            tc.tile_pool(name="const", bufs=1) as cpool,
            tc.tile_pool(name="wpool", bufs=1) as wpool,
            tc.tile_pool(name="hbuf", bufs=1) as hpool,
            tc.tile_pool(name="proj", bufs=3) as ppool,
            tc.tile_pool(name="edge", bufs=3) as epool,
            tc.tile_pool(name="small", bufs=4) as spool,
            tc.tile_pool(name="psA", bufs=2, space="PSUM") as psA,
            tc.tile_pool(name="psZ", bufs=2, space="PSUM") as psZ,
            tc.tile_pool(name="psT", bufs=1, space="PSUM") as psT,
            tc.tile_pool(name="psB", bufs=2, space="PSUM") as psB,
            tc.tile_pool(name="psG", bufs=1, space="PSUM") as psG,
        ):
            nc.gpsimd.load_library(library_config.mlp)

            # ---------- resident constants
            iota_t = cpool.tile([128, 128], F16)
            nc.sync.dma_start(iota_t[:], iota[:])
            piota_t = cpool.tile([128, 1], F32)
            nc.sync.dma_start(piota_t[:], piota[:])
            giota_t = cpool.tile([128, G], F16)
            nc.sync.dma_start(giota_t[:], giota[:])
            ident_t = cpool.tile([128, 128], F16)
            nc.sync.dma_start(ident_t[:], ident[:])
            a_t = {l: cpool.tile([128, HD], F16, tag=f"a{l}", name=f"a{l}_t") for l in (1, 2, 3)}
            for l in (1, 2, 3):
                nc.sync.dma_start(a_t[l][:], a_rep[l][:])
            b_t = {l: cpool.tile([128, HD], F16, tag=f"b{l}", name=f"b{l}_t") for l in (1, 2)}
            for l in (1, 2):
                nc.sync.dma_start(b_t[l][:], b_rep[l][:])
            b3m_t = cpool.tile([128, D], F16)
            nc.sync.dma_start(b3m_t[:], b3m_rep[:])
            x_fm_t = cpool.tile([F_IN, NPC], F16)
            nc.sync.dma_start(x_fm_t[:], x_fm[:])
            idxA_t = cpool.tile([128, NT * FA], I16)
            nc.sync.dma_start(idxA_t[:], idxA[:])
            if KB:
                idxB_t = cpool.tile([128, NT * FB], I16)
                nc.sync.dma_start(idxB_t[:], idxB[:])
            dstrF = cpool.tile([128, NT, EC], F32)
            nc.sync.dma_start(dstrF[:], dstr_f[:].rearrange("t p e -> p t e"))
            gidF = cpool.tile([128, NT], F32)
            nc.sync.dma_start(gidF[:], gid_f[:].rearrange("t p o -> p (t o)"))

            Wt = {}
            for l in (1, 2, 3):
                kdim = F_IN if l == 1 else HD
                chs = [(0, kdim)] if kdim <= 128 else FCH
                Wt[l] = []
                for ci, (off, sz) in enumerate(chs):
                    ws = wpool.tile([sz, TE], F16, tag=f"W{l}s{ci}", name=f"W{l}s{ci}_t")
                    wd = wpool.tile([sz, HD], F16, tag=f"W{l}d{ci}", name=f"W{l}d{ci}_t")
                    nc.sync.dma_start(ws[:], Wmat[l][0][off : off + sz, :])
                    nc.sync.dma_start(wd[:], Wmat[l][1][off : off + sz, :])
                    Wt[l].append((ws, wd))

            hfm = {
                l: [
                    hpool.tile([sz, NPC], F16, tag=f"h{l}fm{ci}", name=f"h{l}fm{ci}")
                    for ci, (off, sz) in enumerate(FCH)
                ]
                for l in (2, 3)
            }
            fd_res = cpool.tile([128, NT, HD], F16)

            gp_ps = psG.tile([G, 65], F32, space="PSUM")

            for l in (1, 2, 3):
                # ---- projection
                for t in range(NT):
                    ps_fs = psA.tile([128, TE], F32, space="PSUM", tag="psP", name="ps_fs")
                    ps_fd = psA.tile([128, HD], F32, space="PSUM", tag="psP", name="ps_fd")
                    if l == 1:
                        lhs = [x_fm_t[:, bass.ts(t, 128)]]
                    else:
                        lhs = [hfm[l][ci][:, bass.ts(t, 128)] for ci in range(len(FCH))]
                    for ci, lt in enumerate(lhs):
                        nc.tensor.matmul(
                            ps_fs[:], lhsT=lt, rhs=Wt[l][ci][0][:],
                            start=(ci == 0), stop=(ci == len(lhs) - 1),
                        )
                    for ci, lt in enumerate(lhs):
                        nc.tensor.matmul(
                            ps_fd[:], lhsT=lt, rhs=Wt[l][ci][1][:],
                            start=(ci == 0), stop=(ci == len(lhs) - 1),
                        )
                    fs_sb = ppool.tile([128, TE], F16, tag="fs_sb")
                    nc.scalar.copy(fs_sb[:], ps_fs[:])
                    nc.scalar.copy(fd_res[:, t, :], ps_fd[:])
                    nc.sync.dma_start(fs_own[l][bass.ts(t, 128), :], fs_sb[:])

                # ---- all-gather fs table
                nc.gpsimd.collective_compute(
                    "AllGather",
                    mybir.AluOpType.bypass,
                    replica_groups=groups,
                    ins=[fs_own[l][:].rearrange("a b -> (a b)")],
                    outs=[fs_full[l][:].rearrange("a b -> (a b)")],
                )

                # ---- edge phase
                for t in range(NT):
                    A = epool.tile([128, EC, TE], F16, tag="A")
                    nc.gpsimd.dma_gather(
                        out_ap=A[:, :ECA, :],
                        in_ap=fs_full[l][:SPLIT, :],
                        idxs_ap=idxA_t[:, t * FA : (t + 1) * FA],
                        num_idxs=KA,
                        num_idxs_reg=KA,
                        elem_size=TE,
                    )
                    if KB:
                        nc.gpsimd.dma_gather(
                            out_ap=A[:, ECA:, :],
                            in_ap=fs_full[l][SPLIT:, :],
                            idxs_ap=idxB_t[:, t * FB : (t + 1) * FB],
                            num_idxs=KB,
                            num_idxs_reg=KB,
                            elem_size=TE,
                        )
                    # node-major one-hot: S_nm[p, e] = (dstr_row[e] == p)
                    drow = spool.tile([1, ET], F32, tag="drow")
                    nc.sync.dma_start(drow[:], dstr_row[t, None, :])
                    dbc = epool.tile([128, ET], F32, tag="dbc")
                    nc.gpsimd.partition_broadcast(dbc[:], drow[:])
                    snm = epool.tile([128, ET], F16, tag="snm")
                    nc.vector.tensor_scalar(
                        out=snm[:], in0=dbc[:], scalar1=piota_t[:],
                        scalar2=None, op0=mybir.AluOpType.is_equal,
                    )
                    # z = fd[dst] + fs, chunk-pairs in PSUM; leaky -> C
                    C = epool.tile([128, EC, HD], F16, tag="C")
                    for j0 in range(0, EC, 2):
                        jn = min(2, EC - j0)
                        zps = psZ.tile([128, 2, HD], F32, space="PSUM", tag="zps", name="zps")
                        for j in range(j0, j0 + jn):
                            nc.tensor.matmul(
                                zps[:, j - j0, :],
                                lhsT=snm[:, bass.ts(j, 128)],
                                rhs=fd_res[:, t, :],
                                start=True, stop=False,
                            )
                            nc.tensor.matmul(
                                zps[:, j - j0, :],
                                lhsT=ident_t[:],
                                rhs=A[:, j, :HD],
                                start=False, stop=True,
                            )
                        nc.scalar.activation(
                            C[:, j0 : j0 + jn, :],
                            zps[:, :jn, :],
                            mybir.ActivationFunctionType.Prelu,
                            alpha=NEG_GAT,
                        )
                    # logits
                    AM = epool.tile([128, EC, HD], F16, tag="AM")
                    nc.vector.tensor_tensor(
                        out=AM[:], in0=C[:],
                        in1=a_t[l][:, None, :].to_broadcast([128, EC, HD]),
                        op=mybir.AluOpType.mult,
                    )
                    logit = spool.tile([128, EC, H], F32, tag="logit")
                    nc.vector.tensor_reduce(
                        out=logit[:],
                        in_=AM[:].rearrange("p a (h d) -> p a h d", h=H),
                        axis=mybir.AxisListType.X,
                        op=mybir.AluOpType.add,
                    )
                    EFX = epool.tile([128, EC, HD + H], F16, tag="EFX")
                    ex = EFX[:, :, HD : HD + H]
                    nc.scalar.activation(
                        ex, logit[:], mybir.ActivationFunctionType.Exp
                    )
                    nc.vector.tensor_tensor(
                        out=EFX[:, :, :HD].rearrange("p a (h d) -> p a h d", h=H),
                        in0=A[:, :, :HD].rearrange("p a (h d) -> p a h d", h=H),
                        in1=ex[:, :, :, None].to_broadcast([128, EC, H, D]),
                        op=mybir.AluOpType.mult,
                    )
                    ps_ud = psB.tile([128, HD + H], F32, space="PSUM", tag="ps_ud", name="ps_ud")
                    for j in range(EC):
                        st = spool.tile([128, 128], F16, tag="st")
                        nc.vector.tensor_scalar(
                            out=st[:], in0=iota_t[:],
                            scalar1=dstrF[:, t, j, None], scalar2=None,
                            op0=mybir.AluOpType.is_equal,
                        )
                        nc.tensor.matmul(
                            ps_ud[:], lhsT=st[:], rhs=EFX[:, j, :],
                            start=(j == 0), stop=(j == EC - 1),
                        )
                    dmax = spool.tile([128, H], F32, tag="dmax")
                    nc.vector.tensor_scalar_max(dmax[:], ps_ud[:, HD : HD + H], 1e-9)
                    rden = spool.tile([128, H], F32, tag="rden")
                    nc.vector.reciprocal(rden[:], dmax[:])
                    hm = spool.tile([128, H, D], F16, tag="hm")
                    nc.vector.tensor_tensor(
                        out=hm[:],
                        in0=ps_ud[:, :HD].rearrange("p (h d) -> p h d", h=H),
                        in1=rden[:, :, None].to_broadcast([128, H, D]),
                        op=mybir.AluOpType.mult,
                    )
                    if l < 3:
                        ht = ppool.tile([128, HD], F16, tag="ht")
                        nc.vector.tensor_tensor(
                            out=ht[:],
                            in0=hm[:].rearrange("p h d -> p (h d)"),
                            in1=b_t[l][:],
                            op=mybir.AluOpType.add,
                        )
                        for ci, (off, sz) in enumerate(FCH):
                            tp = psT.tile([128, 128], F16, space="PSUM", tag="tp", name="tp")
                            nc.tensor.transpose(
                                tp[:sz, :], ht[:, off : off + sz], ident_t[:]
                            )
                            nc.scalar.copy(hfm[l + 1][ci][:, bass.ts(t, 128)], tp[:sz, :])
                    else:
                        rhs65 = ppool.tile([128, 65], F16, tag="rhs65")
                        t01 = spool.tile([128, D], F16, tag="t01")
                        nc.vector.tensor_tensor(
                            out=t01[:], in0=hm[:, 0, :], in1=hm[:, 1, :],
                            op=mybir.AluOpType.add,
                        )
                        t012 = spool.tile([128, D], F16, tag="t012")
                        nc.vector.tensor_tensor(
                            out=t012[:], in0=t01[:], in1=hm[:, 2, :],
                            op=mybir.AluOpType.add,
                        )
                        nc.vector.scalar_tensor_tensor(
                            out=rhs65[:, :D], in0=t012[:], scalar=1.0 / H,
                            in1=b3m_t[:], op0=mybir.AluOpType.mult,
                            op1=mybir.AluOpType.add,
                        )
                        nc.vector.memset(rhs65[:, 64:65], 1.0)
                        gsel = spool.tile([128, G], F16, tag="gsel")
                        nc.vector.tensor_scalar(
                            out=gsel[:], in0=giota_t[:],
                            scalar1=gidF[:, t, None], scalar2=None,
                            op0=mybir.AluOpType.is_equal,
                        )
                        nc.tensor.matmul(
                            gp_ps[:], lhsT=gsel[:], rhs=rhs65[:],
                            start=(t == 0), stop=(t == NT - 1),
                        )

            # ================= epilogue
            part_sb = spool.tile([G, 65], F32, tag="part_sb")
            nc.vector.tensor_copy(part_sb[:], gp_ps[:])
            nc.sync.dma_start(partials[:], part_sb[:])
            nc.gpsimd.collective_compute(
                "AllReduce",
                mybir.AluOpType.add,
                replica_groups=groups,
                ins=[partials[:]],
                outs=[partials_red[:]],
            )
            red_sb = spool.tile([G, 65], F32, tag="red_sb")
            nc.sync.dma_start(red_sb[:], partials_red[:])

            xg = spool.tile([G, 128], F16, tag="xg")
            rc = spool.tile([G, 1], F32, tag="rc")
            cnt1 = spool.tile([G, 1], F32, tag="cnt1")
            nc.vector.tensor_scalar_max(cnt1[:], red_sb[:, 64:65], 1.0)
            nc.vector.reciprocal(rc[:], cnt1[:])
            nc.vector.tensor_tensor(
                out=xg[:, :64], in0=red_sb[:, :64],
                in1=rc[:].to_broadcast([G, 64]), op=mybir.AluOpType.mult,
            )

            px_ps = psA.tile([G, 96], F32, space="PSUM", tag="psP", name="px_ps")
            Wex_t = spool.tile([64, 32], F16, tag="Wex_t")
            nc.sync.dma_start(Wex_t[:], Wex[:])
            for i in range(3):
                pT = spool.tile([64, G], F16, tag=f"pT{i}", name=f"pT{i}")
                nc.sync.dma_start(pT[:], p123T[i][:])
                nc.tensor.matmul(
                    px_ps[:, 32 * i : 32 * i + 32], lhsT=pT[:], rhs=Wex_t[:],
                    start=True, stop=True,
                )
            bex_t = spool.tile([G, 96], F16, tag="bex_t")
            nc.sync.dma_start(bex_t[:], bex96_rep[:])
            pxc = spool.tile([G, 96], F16, tag="pxc")
            nc.vector.tensor_tensor(
                out=pxc[:], in0=px_ps[:], in1=bex_t[:], op=mybir.AluOpType.add
            )

            def small_mm(x_sb, pdim, w_t, b_t_, odim, leaky, out_ap, out_f32=False):
                tp = psT.tile([128, 128], F16, space="PSUM", tag="tp", name="ep_tp")
                nc.tensor.transpose(tp[:pdim, :G], x_sb[:, :pdim], ident_t[:G, :G])
                xT = spool.tile([128, G], F16, tag="ep_xT")
                nc.scalar.copy(xT[:pdim, :], tp[:pdim, :G])
                mm = psA.tile([G, 64], F32, space="PSUM", tag="psP", name="ep_mm")
                nc.tensor.matmul(
                    mm[:, :odim], lhsT=xT[:pdim, :], rhs=w_t[:], start=True, stop=True
                )
                tmp = spool.tile([G, 64], F32 if out_f32 else F16, tag="ep_tmp")
                nc.vector.tensor_tensor(
                    out=tmp[:, :odim], in0=mm[:, :odim], in1=b_t_[:],
                    op=mybir.AluOpType.add,
                )
                if leaky:
                    nc.vector.scalar_tensor_tensor(
                        out=out_ap, in0=tmp[:, :odim], scalar=NEG,
                        in1=tmp[:, :odim], op0=mybir.AluOpType.mult,
                        op1=mybir.AluOpType.max,
                    )
                else:
                    nc.vector.tensor_copy(out_ap, tmp[:, :odim])

            Wpat_t = spool.tile([96, 64], F16, tag="Wpat_t")
            nc.sync.dma_start(Wpat_t[:], Wpat[:])
            bpat_t = spool.tile([G, 64], F16, tag="bpat_t")
            nc.sync.dma_start(bpat_t[:], bpat_rep[:])
            small_mm(pxc, 96, Wpat_t, bpat_t, 64, True, xg[:, 64:128])

            Wc1_t = spool.tile([128, 64], F16, tag="Wc1_t")
            nc.sync.dma_start(Wc1_t[:], Wc1[:])
            bc1_t = spool.tile([G, 64], F16, tag="bc1_t")
            nc.sync.dma_start(bc1_t[:], bc1_rep[:])
            h1 = spool.tile([G, 64], F16, tag="ep_h1")
            small_mm(xg, 128, Wc1_t, bc1_t, 64, True, h1[:])

            Wc2_t = spool.tile([64, 32], F16, tag="Wc2_t")
            nc.sync.dma_start(Wc2_t[:], Wc2[:])
            bc2_t = spool.tile([G, 32], F16, tag="bc2_t")
            nc.sync.dma_start(bc2_t[:], bc2_rep[:])
            h2 = spool.tile([G, 32], F16, tag="ep_h2")
            small_mm(h1, 64, Wc2_t, bc2_t, 32, True, h2[:])

            Wc3_t = spool.tile([32, 2], F16, tag="Wc3_t")
            nc.sync.dma_start(Wc3_t[:], Wc3[:])
            bc3_t = spool.tile([G, 2], F16, tag="bc3_t")
            nc.sync.dma_start(bc3_t[:], bc3_rep[:])
            h3 = spool.tile([G, 2], F32, tag="ep_h3")
            small_mm(h2, 32, Wc3_t, bc3_t, 2, False, h3[:], out_f32=True)
            nc.sync.dma_start(out[:], h3[:])

    nc.finalize()
    return nc


# ---------------------------------------------------------------- entry point

def _run(inputs, trace=False, **trace_kwargs):
    cfg = _derive(_default_cfg())
    in_maps, (KA, KB) = prep_host(inputs, cfg)
    nc = build_gat(cfg, KA, KB)
    res = run_bass_kernel_spmd(
        nc, in_maps, core_ids=list(range(cfg["NC"])), trace=trace, **trace_kwargs
    )
    return np.asarray(res.results[0]["out"], np.float32), res


def kernel(**inputs):
    out, _ = _run(inputs, trace=False)
    return out


# revision 11
# speedup vs baseline: 1.4851x; 1.4851x over previous
"""GATv2 GNN classifier (nn_AttGNNClassifier) as an 8-core Trainium2 Bass kernel.

Strategy (graph-parallel):
  - Nodes are partitioned contiguously across 8 cores (NPC nodes/core, padded).
  - Edges are assigned to the core owning their dst node, grouped into tiles of
    128 dst nodes, and padded to a uniform per-tile edge count (ET = EC*128).
  - Per layer: each core projects its own nodes (fs = h @ Ws, fd = h @ Wd),
    all-gathers fs into a full table (rows padded to 256 fp16 = 512B for the
    gather engine), then processes its edge tiles:
      * dma_gather fs[src] rows (edge-major, int16 indices, split into two
        sections so indices stay < 32768)
      * fd[dst] broadcast to edges via a node-major one-hot matmul, fs added
        into the same PSUM via an identity matmul -> z = fs+fd in PSUM
      * leaky_relu, attention logits via a-mult + head-reduce, exp on ACT,
        and a one-hot (dst == node) matmul aggregation computing unnormalized
        sums and softmax denominators in one PSUM pass.
      * finalize: h = u/denom + bias (head-mean on the last layer).
  - Graph mean-pool via a one-hot (graph_id == g) matmul accumulated across
    tiles, an all-reduce of [G, 65] partials, then the tiny pattern/classifier
    MLP on every core; core 0's output is returned.

Everything data-dependent (edge sorting, padding, index layouts) is prepared on
the host inside kernel() before compiling; the device program is static.
"""

import math

import numpy as np

import concourse.bass as bass
import concourse.bacc as bacc
import concourse.mybir as mybir
import concourse.tile as tile
from concourse import library_config
from concourse.bass_utils import run_bass_kernel_spmd

F16 = mybir.dt.float16
F32 = mybir.dt.float32
I16 = mybir.dt.int16

NEG_GAT = 0.2
NEG = 0.01


def _default_cfg():
    return dict(
        NC=8, N=50000, E=400000, F_IN=128, H=3, D=64, G=64, P=64, SPLIT=32768,
    )


def _derive(cfg):
    c = dict(cfg)
    c["HD"] = c["H"] * c["D"]
    c["TE"] = int(math.ceil(c["HD"] / 128)) * 128  # table row elems (512B rows)
    c["NPC"] = int(math.ceil(c["N"] / c["NC"] / 128)) * 128
    c["NPAD"] = c["NC"] * c["NPC"]
    c["NT"] = c["NPC"] // 128
    fc = []
    off = 0
    while off < c["HD"]:
        sz = min(128, c["HD"] - off)
        fc.append((off, sz))
        off += sz
    c["FCH"] = fc
    assert c["F_IN"] <= 128
    return c


# ---------------------------------------------------------------- host prep

def _wrap16(vals, F):
    """int16 values -> [128, F] wrapped (k -> [k%16, k//16]) x8 replicated."""
    out = np.zeros((128, F), np.int16)
    k = np.arange(len(vals))
    out[k % 16, k // 16] = vals
    for g in range(1, 8):
        out[16 * g : 16 * g + 16] = out[:16]
    return out


def prep_host(inputs, cfg):
    c = cfg
    NC, N, NPC, NT, G = c["NC"], c["N"], c["NPC"], c["NT"], c["G"]
    H, D, HD, F_IN, TE, SPLIT = c["H"], c["D"], c["HD"], c["F_IN"], c["TE"], c["SPLIT"]

    src = np.asarray(inputs["src"]).astype(np.int64)
    dst = np.asarray(inputs["dst"]).astype(np.int64)
    graph_ids = np.asarray(inputs["graph_ids"]).astype(np.int64)
    x = np.asarray(inputs["inputs"]).astype(np.float32)

    owner = dst // NPC
    dstl = dst - owner * NPC
    tile_id = dstl // 128
    dst_rel = dstl % 128
    sect = (src >= SPLIT).astype(np.int64)  # 0 = A, 1 = B

    key = (owner * NT + tile_id) * 2 + sect
    order = np.argsort(key, kind="stable")
    cnt = np.bincount(key[order], minlength=NC * NT * 2).reshape(NC, NT, 2)
    KA = int(math.ceil(max(1, cnt[:, :, 0].max()) / 128)) * 128
    KBraw = int(cnt[:, :, 1].max())
    KB = int(math.ceil(KBraw / 128)) * 128 if KBraw > 0 else 0
    ET = KA + KB
    EC = ET // 128

    dstr_f = np.full((NC, NT, 128, EC), 200.0, np.float16)
    dstr_row = np.full((NC, NT, ET), 200.0, np.float16)
    idxA = np.zeros((NC, NT, KA), np.int64)
    idxB = np.zeros((NC, NT, KB), np.int64) if KB else None

    starts = np.concatenate([[0], np.cumsum(cnt.reshape(-1))]).astype(np.int64)
    for core in range(NC):
        for t in range(NT):
            for s in range(2):
                k = (core * NT + t) * 2 + s
                lo, hi = starts[k], starts[k + 1]
                e = order[lo:hi]
                n = hi - lo
                if n == 0:
                    continue
                base = 0 if s == 0 else KA
                sl = base + np.arange(n)
                p, j = sl % 128, sl // 128
                dstr_f[core, t, p, j] = dst_rel[e]
                dstr_row[core, t, sl] = dst_rel[e]
                if s == 0:
                    idxA[core, t, :n] = src[e]
                else:
                    idxB[core, t, :n] = src[e] - SPLIT

    FA = KA // 16
    idxA_rep = np.zeros((NC, 128, NT * FA), np.int16)
    for core in range(NC):
        for t in range(NT):
            idxA_rep[core, :, t * FA : (t + 1) * FA] = _wrap16(idxA[core, t], FA)
    idxB_rep = None
    if KB:
        FB = KB // 16
        idxB_rep = np.zeros((NC, 128, NT * FB), np.int16)
        for core in range(NC):
            for t in range(NT):
                idxB_rep[core, :, t * FB : (t + 1) * FB] = _wrap16(idxB[core, t], FB)

    x_fm = np.zeros((NC, F_IN, NPC), np.float16)
    for core in range(NC):
        lo, hi = core * NPC, min(N, (core + 1) * NPC)
        if hi > lo:
            x_fm[core, :, : hi - lo] = x[lo:hi].T.astype(np.float16)

    gid_f = np.full((NC, NT, 128), -1.0, np.float32)
    for core in range(NC):
        lo, hi = core * NPC, min(N, (core + 1) * NPC)
        if hi > lo:
            flat = np.full(NPC, -1.0, np.float32)
            flat[: hi - lo] = graph_ids[lo:hi].astype(np.float32)
            gid_f[core] = flat.reshape(NT, 128)

    rep = lambda v, p=128: np.broadcast_to(
        np.asarray(v, np.float16)[None, :], (p, len(v))
    ).copy()

    def w16(k):
        return np.asarray(inputs[k]).astype(np.float16)

    def ws_pad(k):  # pad output cols HD -> TE
        w = np.asarray(inputs[k]).astype(np.float16)
        out = np.zeros((w.shape[0], TE), np.float16)
        out[:, :HD] = w
        return out

    a_flat = [np.asarray(inputs[f"a{l}"]).astype(np.float32).reshape(-1) for l in (1, 2, 3)]
    b_flat = [np.asarray(inputs[f"b{l}"]).astype(np.float32) for l in (1, 2, 3)]
    b3m = b_flat[2].reshape(H, D).mean(0)

    iota128 = np.broadcast_to(np.arange(128, dtype=np.float16)[None, :], (128, 128)).copy()
    piota = np.arange(128, dtype=np.float32)[:, None].copy()
    giota = np.broadcast_to(np.arange(G, dtype=np.float16)[None, :], (128, G)).copy()
    ident = np.eye(128, dtype=np.float16)

    bex = np.asarray(inputs["bex"]).astype(np.float32)
    bex96 = np.concatenate([bex, bex, bex])

    common = dict(
        W1s=ws_pad("W1s"), W1d=w16("W1d"),
        W2s=ws_pad("W2s"), W2d=w16("W2d"),
        W3s=ws_pad("W3s"), W3d=w16("W3d"),
        a1_rep=rep(a_flat[0]), a2_rep=rep(a_flat[1]), a3_rep=rep(a_flat[2]),
        b1_rep=rep(b_flat[0]), b2_rep=rep(b_flat[1]),
        b3m_rep=rep(b3m),
        iota=iota128, piota=piota, giota=giota, ident=ident,
        p1T=w16("p1").T.copy(), p2T=w16("p2").T.copy(), p3T=w16("p3").T.copy(),
        Wex=w16("Wex"), bex96_rep=rep(bex96, G),
        Wpat=w16("Wpat"), bpat_rep=rep(np.asarray(inputs["bpat"], np.float32), G),
        Wc1=w16("Wc1"), bc1_rep=rep(np.asarray(inputs["bc1"], np.float32), G),
        Wc2=w16("Wc2"), bc2_rep=rep(np.asarray(inputs["bc2"], np.float32), G),
        Wc3=w16("Wc3"), bc3_rep=rep(np.asarray(inputs["bc3"], np.float32), G),
    )

    in_maps = []
    for core in range(NC):
        m = dict(common)
        m["x_fm"] = x_fm[core]
        m["idxA"] = idxA_rep[core]
        if KB:
            m["idxB"] = idxB_rep[core]
        m["dstr_f"] = dstr_f[core]
        m["dstr_row"] = dstr_row[core]
        m["gid_f"] = gid_f[core][:, :, None]
        in_maps.append(m)
    return in_maps, (KA, KB)


# ---------------------------------------------------------------- device build

def build_gat(cfg, KA, KB):
    c = cfg
    NC, NPC, NPAD, NT, G = c["NC"], c["NPC"], c["NPAD"], c["NT"], c["G"]
    H, D, HD, F_IN, TE, SPLIT = c["H"], c["D"], c["HD"], c["F_IN"], c["TE"], c["SPLIT"]
    FCH = c["FCH"]
    ET = KA + KB
    EC, ECA = ET // 128, KA // 128
    FA, FB = KA // 16, KB // 16

    nc = bacc.Bacc("TRN2", target_bir_lowering=False, debug=False, num_devices=NC,
                   num_swdge_queues=4)

    def din(name, shape, dt=F16):
        return nc.dram_tensor(name, shape, dt, kind="ExternalInput")

    x_fm = din("x_fm", [F_IN, NPC])
    idxA = din("idxA", [128, NT * FA], I16)
    idxB = din("idxB", [128, NT * FB], I16) if KB else None
    dstr_f = din("dstr_f", [NT, 128, EC], F16)
    dstr_row = din("dstr_row", [NT, ET], F16)
    gid_f = din("gid_f", [NT, 128, 1], F32)

    Wmat = {
        1: (din("W1s", [F_IN, TE]), din("W1d", [F_IN, HD])),
        2: (din("W2s", [HD, TE]), din("W2d", [HD, HD])),
        3: (din("W3s", [HD, TE]), din("W3d", [HD, HD])),
    }
    a_rep = {l: din(f"a{l}_rep", [128, HD]) for l in (1, 2, 3)}
    b_rep = {1: din("b1_rep", [128, HD]), 2: din("b2_rep", [128, HD])}
    b3m_rep = din("b3m_rep", [128, D])
    iota = din("iota", [128, 128])
    piota = din("piota", [128, 1], F32)
    giota = din("giota", [128, G])
    ident = din("ident", [128, 128])
    p123T = [din("p1T", [64, G]), din("p2T", [64, G]), din("p3T", [64, G])]
    Wex = din("Wex", [64, 32])
    bex96_rep = din("bex96_rep", [G, 96])
    Wpat = din("Wpat", [96, 64])
    bpat_rep = din("bpat_rep", [G, 64])
    Wc1 = din("Wc1", [128, 64])
    bc1_rep = din("bc1_rep", [G, 64])
    Wc2 = din("Wc2", [64, 32])
    bc2_rep = din("bc2_rep", [G, 32])
    Wc3 = din("Wc3", [32, 2])
    bc3_rep = din("bc3_rep", [G, 2])

    out = nc.dram_tensor("out", [G, 2], F32, kind="ExternalOutput")

    fs_own = {l: nc.dram_tensor(f"fs_own{l}", [NPC, TE], F16) for l in (1, 2, 3)}
    fs_full = {
        l: nc.dram_tensor(f"fs_full{l}", [NPAD, TE], F16, addr_space="Shared")
        for l in (1, 2, 3)
    }
    partials = nc.dram_tensor("partials", [G, 65], F32)
    partials_red = nc.dram_tensor("partials_red", [G, 65], F32, addr_space="Shared")

    groups = [list(range(NC))]

    with tile.TileContext(nc) as tc:
        with (
            tc.tile_pool(name="const", bufs=1) as cpool,
            tc.tile_pool(name="wpool", bufs=1) as wpool,
            tc.tile_pool(name="hbuf", bufs=1) as hpool,
            tc.tile_pool(name="proj", bufs=3) as ppool,
            tc.tile_pool(name="edge", bufs=2) as epool,
            tc.tile_pool(name="small", bufs=2) as spool,
            tc.tile_pool(name="psA", bufs=2, space="PSUM") as psA,
            tc.tile_pool(name="psZ", bufs=2, space="PSUM") as psZ,
            tc.tile_pool(name="psT", bufs=1, space="PSUM") as psT,
            tc.tile_pool(name="psB", bufs=2, space="PSUM") as psB,
            tc.tile_pool(name="psG", bufs=1, space="PSUM") as psG,
        ):
            nc.gpsimd.load_library(library_config.mlp)

            # ---------- resident constants
            iota_t = cpool.tile([128, 128], F16)
            nc.sync.dma_start(iota_t[:], iota[:])
            piota_t = cpool.tile([128, 1], F32)
            nc.sync.dma_start(piota_t[:], piota[:])
            giota_t = cpool.tile([128, G], F16)
            nc.sync.dma_start(giota_t[:], giota[:])
            ident_t = cpool.tile([128, 128], F16)
            nc.sync.dma_start(ident_t[:], ident[:])
            a_t = {l: cpool.tile([128, HD], F16, tag=f"a{l}", name=f"a{l}_t") for l in (1, 2, 3)}
            for l in (1, 2, 3):
                nc.sync.dma_start(a_t[l][:], a_rep[l][:])
            b_t = {l: cpool.tile([128, HD], F16, tag=f"b{l}", name=f"b{l}_t") for l in (1, 2)}
            for l in (1, 2):
                nc.sync.dma_start(b_t[l][:], b_rep[l][:])
            b3m_t = cpool.tile([128, D], F16)
            nc.sync.dma_start(b3m_t[:], b3m_rep[:])
            x_fm_t = cpool.tile([F_IN, NPC], F16)
            nc.sync.dma_start(x_fm_t[:], x_fm[:])
            idxA_t = cpool.tile([128, NT * FA], I16)
            nc.sync.dma_start(idxA_t[:], idxA[:])
            if KB:
                idxB_t = cpool.tile([128, NT * FB], I16)
                nc.sync.dma_start(idxB_t[:], idxB[:])
            dstrF = cpool.tile([128, NT, EC], F16)
            nc.sync.dma_start(dstrF[:], dstr_f[:].rearrange("t p e -> p t e"))
            gidF = cpool.tile([128, NT], F32)
            nc.sync.dma_start(gidF[:], gid_f[:].rearrange("t p o -> p (t o)"))

            Wt = {}
            for l in (1, 2, 3):
                kdim = F_IN if l == 1 else HD
                chs = [(0, kdim)] if kdim <= 128 else FCH
                Wt[l] = []
                for ci, (off, sz) in enumerate(chs):
                    ws = wpool.tile([sz, TE], F16, tag=f"W{l}s{ci}", name=f"W{l}s{ci}_t")
                    wd = wpool.tile([sz, HD], F16, tag=f"W{l}d{ci}", name=f"W{l}d{ci}_t")
                    nc.sync.dma_start(ws[:], Wmat[l][0][off : off + sz, :])
                    nc.sync.dma_start(wd[:], Wmat[l][1][off : off + sz, :])
                    Wt[l].append((ws, wd))

            hfm = {
                l: [
                    hpool.tile([sz, NPC], F16, tag=f"h{l}fm{ci}", name=f"h{l}fm{ci}")
                    for ci, (off, sz) in enumerate(FCH)
                ]
                for l in (2, 3)
            }
            fd_res = cpool.tile([128, NT, HD], F16)

            gp_ps = psG.tile([G, 65], F32, space="PSUM")

            for l in (1, 2, 3):
                # ---- projection
                for t in range(NT):
                    ps_fs = psA.tile([128, TE], F32, space="PSUM", tag="psP", name="ps_fs")
                    ps_fd = psA.tile([128, HD], F32, space="PSUM", tag="psP", name="ps_fd")
                    if l == 1:
                        lhs = [x_fm_t[:, bass.ts(t, 128)]]
                    else:
                        lhs = [hfm[l][ci][:, bass.ts(t, 128)] for ci in range(len(FCH))]
                    for ci, lt in enumerate(lhs):
                        nc.tensor.matmul(
                            ps_fs[:], lhsT=lt, rhs=Wt[l][ci][0][:],
                            start=(ci == 0), stop=(ci == len(lhs) - 1),
                        )
                    for ci, lt in enumerate(lhs):
                        nc.tensor.matmul(
                            ps_fd[:], lhsT=lt, rhs=Wt[l][ci][1][:],
                            start=(ci == 0), stop=(ci == len(lhs) - 1),
                        )
                    fs_sb = ppool.tile([128, TE], F16, tag="fs_sb")
                    nc.scalar.copy(fs_sb[:], ps_fs[:])
                    nc.scalar.copy(fd_res[:, t, :], ps_fd[:])
                    nc.sync.dma_start(fs_own[l][bass.ts(t, 128), :], fs_sb[:])

                # ---- all-gather fs table
                nc.gpsimd.collective_compute(
                    "AllGather",
                    mybir.AluOpType.bypass,
                    replica_groups=groups,
                    ins=[fs_own[l][:].rearrange("a b -> (a b)")],
                    outs=[fs_full[l][:].rearrange("a b -> (a b)")],
                )

                # ---- edge phase
                for t in range(NT):
                    A = epool.tile([128, EC, TE], F16, tag="A", bufs=3)
                    nc.gpsimd.dma_gather(
                        out_ap=A[:, :ECA, :],
                        in_ap=fs_full[l][:SPLIT, :],
                        idxs_ap=idxA_t[:, t * FA : (t + 1) * FA],
                        num_idxs=KA,
                        num_idxs_reg=KA,
                        elem_size=TE,
                        queue_num=(2 * t) % 4,
                    )
                    if KB:
                        nc.gpsimd.dma_gather(
                            out_ap=A[:, ECA:, :],
                            in_ap=fs_full[l][SPLIT:, :],
                            idxs_ap=idxB_t[:, t * FB : (t + 1) * FB],
                            num_idxs=KB,
                            num_idxs_reg=KB,
                            elem_size=TE,
                            queue_num=(2 * t + 1) % 4,
                        )
                    # node-major one-hot: S_nm[p, e] = (dstr_row[e] == p)
                    drow = spool.tile([1, ET], F16, tag="drow", bufs=2)
                    nc.sync.dma_start(drow[:], dstr_row[t, None, :])
                    dbc = epool.tile([128, ET], F16, tag="dbc")
                    nc.gpsimd.partition_broadcast(dbc[:], drow[:])
                    snm = epool.tile([128, ET], F16, tag="snm")
                    nc.vector.tensor_scalar(
                        out=snm[:], in0=dbc[:], scalar1=piota_t[:],
                        scalar2=None, op0=mybir.AluOpType.is_equal,
                    )
                    # z = fd[dst] + fs, chunk-pairs in PSUM; leaky -> C
                    C = epool.tile([128, EC, HD], F16, tag="C")
                    for j0 in range(0, EC, 2):
                        jn = min(2, EC - j0)
                        zps = psZ.tile([128, 2, HD], F32, space="PSUM", tag="zps", name="zps")
                        for j in range(j0, j0 + jn):
                            nc.tensor.matmul(
                                zps[:, j - j0, :],
                                lhsT=snm[:, bass.ts(j, 128)],
                                rhs=fd_res[:, t, :],
                                start=True, stop=False,
                            )
                            nc.tensor.matmul(
                                zps[:, j - j0, :],
                                lhsT=ident_t[:],
                                rhs=A[:, j, :HD],
                                start=False, stop=True,
                            )
                        nc.scalar.activation(
                            C[:, j0 : j0 + jn, :],
                            zps[:, :jn, :],
                            mybir.ActivationFunctionType.Prelu,
                            alpha=NEG_GAT,
                        )
                    # logits
                    AM = epool.tile([128, EC, HD], F16, tag="AM")
                    nc.vector.tensor_tensor(
                        out=AM[:], in0=C[:],
                        in1=a_t[l][:, None, :].to_broadcast([128, EC, HD]),
                        op=mybir.AluOpType.mult,
                    )
                    logit = spool.tile([128, EC, H], F32, tag="logit")
                    nc.vector.tensor_reduce(
                        out=logit[:],
                        in_=AM[:].rearrange("p a (h d) -> p a h d", h=H),
                        axis=mybir.AxisListType.X,
                        op=mybir.AluOpType.add,
                    )
                    EFX = epool.tile([128, EC, HD + H], F16, tag="EFX")
                    ex = EFX[:, :, HD : HD + H]
                    nc.scalar.activation(
                        ex, logit[:], mybir.ActivationFunctionType.Exp
                    )
                    nc.vector.tensor_tensor(
                        out=EFX[:, :, :HD].rearrange("p a (h d) -> p a h d", h=H),
                        in0=A[:, :, :HD].rearrange("p a (h d) -> p a h d", h=H),
                        in1=ex[:, :, :, None].to_broadcast([128, EC, H, D]),
                        op=mybir.AluOpType.mult,
                    )
                    ps_ud = psB.tile([128, HD + H], F32, space="PSUM", tag="ps_ud", name="ps_ud")
                    st_all = epool.tile([128, EC, 128], F16, tag="st_all")
                    nc.vector.tensor_tensor(
                        out=st_all[:],
                        in0=iota_t[:, None, :].to_broadcast([128, EC, 128]),
                        in1=dstrF[:, t, :, None].to_broadcast([128, EC, 128]),
                        op=mybir.AluOpType.is_equal,
                    )
                    for j in range(EC):
                        nc.tensor.matmul(
                            ps_ud[:], lhsT=st_all[:, j, :], rhs=EFX[:, j, :],
                            start=(j == 0), stop=(j == EC - 1),
                        )
                    dmax = spool.tile([128, H], F32, tag="dmax")
                    nc.vector.tensor_scalar_max(dmax[:], ps_ud[:, HD : HD + H], 1e-9)
                    rden = spool.tile([128, H], F32, tag="rden")
                    nc.vector.reciprocal(rden[:], dmax[:])
                    hm = spool.tile([128, H, D], F16, tag="hm")
                    nc.vector.tensor_tensor(
                        out=hm[:],
                        in0=ps_ud[:, :HD].rearrange("p (h d) -> p h d", h=H),
                        in1=rden[:, :, None].to_broadcast([128, H, D]),
                        op=mybir.AluOpType.mult,
                    )
                    if l < 3:
                        ht = ppool.tile([128, HD], F16, tag="ht")
                        nc.vector.tensor_tensor(
                            out=ht[:],
                            in0=hm[:].rearrange("p h d -> p (h d)"),
                            in1=b_t[l][:],
                            op=mybir.AluOpType.add,
                        )
                        for ci, (off, sz) in enumerate(FCH):
                            tp = psT.tile([128, 128], F16, space="PSUM", tag="tp", name="tp")
                            nc.tensor.transpose(
                                tp[:sz, :], ht[:, off : off + sz], ident_t[:]
                            )
                            nc.scalar.copy(hfm[l + 1][ci][:, bass.ts(t, 128)], tp[:sz, :])
                    else:
                        rhs65 = ppool.tile([128, 65], F16, tag="rhs65")
                        t01 = spool.tile([128, D], F16, tag="t01")
                        nc.vector.tensor_tensor(
                            out=t01[:], in0=hm[:, 0, :], in1=hm[:, 1, :],
                            op=mybir.AluOpType.add,
                        )
                        t012 = spool.tile([128, D], F16, tag="t012")
                        nc.vector.tensor_tensor(
                            out=t012[:], in0=t01[:], in1=hm[:, 2, :],
                            op=mybir.AluOpType.add,
                        )
                        nc.vector.scalar_tensor_tensor(
                            out=rhs65[:, :D], in0=t012[:], scalar=1.0 / H,
                            in1=b3m_t[:], op0=mybir.AluOpType.mult,
                            op1=mybir.AluOpType.add,
                        )
                        nc.vector.memset(rhs65[:, 64:65], 1.0)
                        gsel = spool.tile([128, G], F16, tag="gsel")
                        nc.vector.tensor_scalar(
                            out=gsel[:], in0=giota_t[:],
                            scalar1=gidF[:, t, None], scalar2=None,
                            op0=mybir.AluOpType.is_equal,
                        )
                        nc.tensor.matmul(
                            gp_ps[:], lhsT=gsel[:], rhs=rhs65[:],
                            start=(t == 0), stop=(t == NT - 1),
                        )

            # ================= epilogue
            part_sb = spool.tile([G, 65], F32, tag="part_sb")
            nc.vector.tensor_copy(part_sb[:], gp_ps[:])
            nc.sync.dma_start(partials[:], part_sb[:])
            nc.gpsimd.collective_compute(
                "AllReduce",
                mybir.AluOpType.add,
                replica_groups=groups,
                ins=[partials[:]],
                outs=[partials_red[:]],
            )
            red_sb = spool.tile([G, 65], F32, tag="red_sb")
            nc.sync.dma_start(red_sb[:], partials_red[:])

            xg = spool.tile([G, 128], F16, tag="xg")
            rc = spool.tile([G, 1], F32, tag="rc")
            cnt1 = spool.tile([G, 1], F32, tag="cnt1")
            nc.vector.tensor_scalar_max(cnt1[:], red_sb[:, 64:65], 1.0)
            nc.vector.reciprocal(rc[:], cnt1[:])
            nc.vector.tensor_tensor(
                out=xg[:, :64], in0=red_sb[:, :64],
                in1=rc[:].to_broadcast([G, 64]), op=mybir.AluOpType.mult,
            )

            px_ps = psA.tile([G, 96], F32, space="PSUM", tag="psP", name="px_ps")
            Wex_t = spool.tile([64, 32], F16, tag="Wex_t")
            nc.sync.dma_start(Wex_t[:], Wex[:])
            for i in range(3):
                pT = spool.tile([64, G], F16, tag=f"pT{i}", name=f"pT{i}")
                nc.sync.dma_start(pT[:], p123T[i][:])
                nc.tensor.matmul(
                    px_ps[:, 32 * i : 32 * i + 32], lhsT=pT[:], rhs=Wex_t[:],
                    start=True, stop=True,
                )
            bex_t = spool.tile([G, 96], F16, tag="bex_t")
            nc.sync.dma_start(bex_t[:], bex96_rep[:])
            pxc = spool.tile([G, 96], F16, tag="pxc")
            nc.vector.tensor_tensor(
                out=pxc[:], in0=px_ps[:], in1=bex_t[:], op=mybir.AluOpType.add
            )

            def small_mm(x_sb, pdim, w_t, b_t_, odim, leaky, out_ap, out_f32=False):
                tp = psT.tile([128, 128], F16, space="PSUM", tag="tp", name="ep_tp")
                nc.tensor.transpose(tp[:pdim, :G], x_sb[:, :pdim], ident_t[:G, :G])
                xT = spool.tile([128, G], F16, tag="ep_xT")
                nc.scalar.copy(xT[:pdim, :], tp[:pdim, :G])
                mm = psA.tile([G, 64], F32, space="PSUM", tag="psP", name="ep_mm")
                nc.tensor.matmul(
                    mm[:, :odim], lhsT=xT[:pdim, :], rhs=w_t[:], start=True, stop=True
                )
                tmp = spool.tile([G, 64], F32 if out_f32 else F16, tag="ep_tmp")
                nc.vector.tensor_tensor(
                    out=tmp[:, :odim], in0=mm[:, :odim], in1=b_t_[:],
                    op=mybir.AluOpType.add,
                )
                if leaky:
                    nc.vector.scalar_tensor_tensor(
                        out=out_ap, in0=tmp[:, :odim], scalar=NEG,
                        in1=tmp[:, :odim], op0=mybir.AluOpType.mult,
                        op1=mybir.AluOpType.max,
                    )
                else:
                    nc.vector.tensor_copy(out_ap, tmp[:, :odim])

            Wpat_t = spool.tile([96, 64], F16, tag="Wpat_t")
            nc.sync.dma_start(Wpat_t[:], Wpat[:])
            bpat_t = spool.tile([G, 64], F16, tag="bpat_t")
            nc.sync.dma_start(bpat_t[:], bpat_rep[:])
            small_mm(pxc, 96, Wpat_t, bpat_t, 64, True, xg[:, 64:128])

            Wc1_t = spool.tile([128, 64], F16, tag="Wc1_t")
            nc.sync.dma_start(Wc1_t[:], Wc1[:])
            bc1_t = spool.tile([G, 64], F16, tag="bc1_t")
            nc.sync.dma_start(bc1_t[:], bc1_rep[:])
            h1 = spool.tile([G, 64], F16, tag="ep_h1")
            small_mm(xg, 128, Wc1_t, bc1_t, 64, True, h1[:])

            Wc2_t = spool.tile([64, 32], F16, tag="Wc2_t")
            nc.sync.dma_start(Wc2_t[:], Wc2[:])
            bc2_t = spool.tile([G, 32], F16, tag="bc2_t")
            nc.sync.dma_start(bc2_t[:], bc2_rep[:])
            h2 = spool.tile([G, 32], F16, tag="ep_h2")
            small_mm(h1, 64, Wc2_t, bc2_t, 32, True, h2[:])

            Wc3_t = spool.tile([32, 2], F16, tag="Wc3_t")
            nc.sync.dma_start(Wc3_t[:], Wc3[:])
            bc3_t = spool.tile([G, 2], F16, tag="bc3_t")
            nc.sync.dma_start(bc3_t[:], bc3_rep[:])
            h3 = spool.tile([G, 2], F32, tag="ep_h3")
            small_mm(h2, 32, Wc3_t, bc3_t, 2, False, h3[:], out_f32=True)
            nc.sync.dma_start(out[:], h3[:])

    nc.finalize()
    return nc


# ---------------------------------------------------------------- entry point

def _run(inputs, trace=False, **trace_kwargs):
    cfg = _derive(_default_cfg())
    in_maps, (KA, KB) = prep_host(inputs, cfg)
    nc = build_gat(cfg, KA, KB)
    res = run_bass_kernel_spmd(
        nc, in_maps, core_ids=list(range(cfg["NC"])), trace=trace, **trace_kwargs
    )
    return np.asarray(res.results[0]["out"], np.float32), res


def kernel(**inputs):
    out, _ = _run(inputs, trace=False)
    return out
